# revision 5
# baseline (speedup 1.0000x reference)
"""Additive attention (nn_AdditiveAttention) Bass kernel for 8 TRN2 NeuronCores.

Reference computation (B=16, Q=64, K=1024, QS=KS=VS=256, H=128):
    q = queries @ Wq                      # (B,Q,H)
    k = keys @ Wk                         # (B,K,H)
    feat = tanh(q[:,:,None,:] + k[:,None,:,:])   # (B,Q,K,H)
    scores = feat @ Ws                    # (B,Q,K)
    scores = where(arange(K) >= valid_len[b], scores, -1e6)
    out = softmax(scores) @ values        # (B,Q,VS)

Strategy: replace the elementwise tanh over (B,Q,K,H) with a rank-R
separable approximation

    tanh(q + k) ~= sum_r w_r(q) * g_r(k)

where the k-side basis g_r is device-computable (clipped powers k^1..k^8
split across DVE and GpSimd, shifted tanh(k+s) on ACT) and the q-side
weights w_r are evaluated EXACTLY on the host (per-q L2 fit from a
lookup table). Then

    scores[q,k] = sum_h Ws_h tanh(qf+kf) ~= sum_r <P_r[:,q], g_r(kf)[:,k]>_h

with P_r[h,q] = Ws_h * w_r(qf[h,q]) shipped bf16 from host: R matmuls
contracting over H=128 replace the 134M-element tanh (ACT-bound in the
old kernel). A constant basis column is included in the fit but dropped
on device: it shifts each row's scores by a per-q constant, which
softmax cancels.

Work split per core (data-parallel over batch, 2 batches/core as slot0
rows 0-63 and slot1 rows 64-127, valid_len-aware skipping of masked
leading keys):
  - host: qf, kf projections (exact fp32 GEMM), P_r tables, masks,
    values slicing/padding, all bf16 casts.
  - device DVE: clip kf, powers t2-t5,t7; GpSimd: t6,t8.
  - device ACT: 3 shifted tanh basis columns, blockwise exp.
  - device PE: R matmuls per (slot, 512-col half) into fp32 PSUM seeded
    with the additive -1e6 valid_len mask, attn transposes, attn @ V.
"""

import sys

if "/opt/trn_rl_repo" not in sys.path:
    sys.path.insert(0, "/opt/trn_rl_repo")

import ml_dtypes
import numpy as np

import concourse.bass as bass  # noqa: F401
import concourse.mybir as mybir
import concourse.tile as tile
from concourse import bacc
from concourse.bass_utils import run_bass_kernel_spmd

LAST_RESULT = None  # BassKernelResults of the most recent kernel() call

B, Q, K = 16, 64, 1024
QS = KS = VS = 256
H = 128
NCORES = 8
NEG = -1.0e6
F32 = mybir.dt.float32
BF16 = mybir.dt.bfloat16
NP_BF16 = ml_dtypes.bfloat16

NPOW = 8                      # clipped powers k^1..k^NPOW
SHIFTS = (-3.0, 0.0, 3.0)     # tanh(k + s) basis columns (ACT)
R = NPOW + len(SHIFTS)        # device basis size (const col dropped)
CLAMP = 4.5
NWARM = 6                     # PE clock-ramp warmup matmuls

_FIT_CACHE = None


def _bf(x):
    return np.asarray(x, np.float32).astype(NP_BF16).astype(np.float32)


def _basis_cols(kv):
    """[len(kv), R+1] host model of the device basis (col 0 = const),
    including the bf16 rounding of the device compute chain."""
    kv = np.asarray(kv, np.float32)
    t1 = _bf(np.clip(kv, -CLAMP, CLAMP))
    cols = [np.ones_like(kv), t1]
    cur = t1
    for _ in range(2, NPOW + 1):
        cur = _bf(cur * t1)
        cols.append(cur)
    for s in SHIFTS:
        cols.append(_bf(np.tanh(kv + s)))
    return np.stack(cols, -1).astype(np.float32)


def _fit_tables():
    """Per-q weight lookup table (qgrid, Wt[nq, R+1]) for the L2 fit of
    tanh(q+k) onto the device basis, under a Gaussian+floor k-weight."""
    global _FIT_CACHE
    if _FIT_CACHE is not None:
        return _FIT_CACHE
    kgrid = np.linspace(-5.6, 5.6, 2241)
    wg = np.exp(-kgrid ** 2 / 2)
    wg /= wg.sum()
    wg += 0.01 / len(kgrid)
    qgrid = np.linspace(-5.2, 5.2, 2081)
    Gk = _basis_cols(kgrid)
    sw = np.sqrt(wg)[:, None]
    gram = (Gk * sw).T @ (Gk * sw) + 1e-6 * np.eye(R + 1)
    T = np.tanh(qgrid[:, None] + kgrid[None, :])
    bm = (T * wg[None, :]) @ Gk
    Wt = np.linalg.solve(gram, bm.T).T
    _FIT_CACHE = (qgrid, Wt)
    return _FIT_CACHE


def _build(L, nblkv):
    """Per-core Bass graph. L/nblkv: 2-element lists of per-slot kept key
    length (multiple of 8, > 512) and value block count (ceil(L/128))."""
    nc = bacc.Bacc("TRN2", target_bir_lowering=False, debug=False,
                   num_devices=NCORES)
    L0, L1 = L
    LT = L0 + L1
    nB = [n - 4 for n in nblkv]
    nBmax = max(nB)

    # chunk meta: (slot, kfT col offset, width, psum half)
    chunks = [
        (0, 0, 512, 0),
        (1, L0, 512, 0),
        (0, 512, L0 - 512, 1),
        (1, L0 + 512, L1 - 512, 1),
    ]

    inp = {
        "kfT4": nc.dram_tensor("kfT4", [4, 128, 512], BF16,
                               kind="ExternalInput").ap(),
        "Pmat": nc.dram_tensor("Pmat", [128, 2 * R * 64], BF16,
                               kind="ExternalInput").ap(),
        "maskAB": nc.dram_tensor("maskAB", [2, 1024], BF16,
                                 kind="ExternalInput").ap(),
        "onesAB": nc.dram_tensor("onesAB", [2, 128], BF16,
                                 kind="ExternalInput").ap(),
        "identb": nc.dram_tensor("identb", [128, 128], BF16,
                                 kind="ExternalInput").ap(),
    }
    for s in range(2):
        inp[f"values{s}"] = nc.dram_tensor(
            f"values{s}", [nblkv[s], 128, VS], BF16,
            kind="ExternalInput").ap()
    out_d = nc.dram_tensor("out", [128, VS], F32, kind="ExternalOutput").ap()

    with tile.TileContext(nc) as tc:
        with (
            tc.tile_pool(name="consts", bufs=1) as consts,
            tc.tile_pool(name="feat", bufs=1) as feat,
            tc.tile_pool(name="vals", bufs=1) as vals,
            tc.tile_pool(name="soft", bufs=1) as soft,
        ):
            # --- early const setup -----------------------------------------
            # warm/bias memsets on GpSimd (earliest-starting engine) so PE
            # warmup + the ACT table preload aren't gated on DVE/DMA
            warm_sb = consts.tile([128, 512], BF16)
            nc.gpsimd.memset(warm_sb, 0.5)
            bias_sb = consts.tile([128, len(SHIFTS)], F32)
            for si, sh in enumerate(SHIFTS):
                nc.gpsimd.memset(bias_sb[:, si:si + 1], float(sh))
            dum_sb = consts.tile([128, 1], BF16)

            # small consts on the gpsimd (SWDGE) queue
            ones_sb = consts.tile([2, 128], BF16)
            nc.gpsimd.dma_start(out=ones_sb, in_=inp["onesAB"])
            mask_sb = consts.tile([2, 1024], BF16)
            nc.gpsimd.dma_start(out=mask_sb, in_=inp["maskAB"])
            identb_sb = consts.tile([128, 128], BF16)
            nc.gpsimd.dma_start(out=identb_sb, in_=inp["identb"])

            # ACT activation-table preload with dummy ops while DMAs land
            nc.scalar.activation(out=dum_sb, in_=bias_sb[:, 0:1],
                                 func=mybir.ActivationFunctionType.Tanh,
                                 bias=bias_sb[:, 0:1])
            nc.scalar.activation(out=dum_sb, in_=bias_sb[:, 0:1],
                                 func=mybir.ActivationFunctionType.Exp)

            # PE warmup (HAM clock-gate ramp) as soon as warm_sb is set
            warmps = tc.alloc_tile_pool(name="warmps", bufs=1, space="PSUM")
            warm_ps = warmps.tile([128, 512], F32)
            for _ in range(NWARM):
                nc.tensor.matmul(warm_ps, warm_sb[:, 0:128], warm_sb,
                                 start=True, stop=True)

            # --- bulk DMAs on hardware queues ------------------------------
            # scalar queue: Pmat first (gates all score matmuls), then the
            # kfT chunks in compute order, then slot1 values.
            p_sb = consts.tile([128, 2 * R * 64], BF16)
            nc.scalar.dma_start(out=p_sb, in_=inp["Pmat"])
            kfT = feat.tile([128, LT], BF16)
            for ci, (s, o, w, half) in enumerate(chunks):
                nc.scalar.dma_start(out=kfT[:, o:o + w],
                                    in_=inp["kfT4"][ci, :, 0:w])
            # values: slot0 on sync queue, slot1 on scalar queue (after kfT)
            vals_sb = []
            for s in range(2):
                v = vals.tile([128, nblkv[s], VS], BF16, name=f"vals{s}")
                eng = nc.sync if s == 0 else nc.scalar
                for j in range(nblkv[s]):
                    eng.dma_start(out=v[:, j, :], in_=inp[f"values{s}"][j])
                vals_sb.append(v)

            def pslice(s, r):
                o = (s * R + r) * 64
                return p_sb[:, o:o + 64]

            # --- basis + scores --------------------------------------------
            tpow = [feat.tile([128, LT], BF16, name=f"t{i + 1}")
                    for i in range(NPOW)]
            ttanh = [feat.tile([128, LT], BF16, name=f"tanh{si}")
                     for si in range(len(SHIFTS))]
            basis = tpow + ttanh

            scps = tc.alloc_tile_pool(name="scps", bufs=1, space="PSUM")
            scA = scps.tile([128, 512], F32, tag="scA")
            scB = scps.tile([128, 512], F32, tag="scB")
            nc.tensor.matmul(scA, ones_sb, mask_sb[:, 0:512], start=True,
                             stop=False)
            nc.tensor.matmul(scB, ones_sb, mask_sb[:, 512:1024], start=True,
                             stop=False)

            expm = soft.tile([128, 1024], BF16)

            def exp_blocks(sc, base, jrange):
                for j in jrange:
                    lo = base * 128 + j * 128
                    nc.scalar.activation(
                        out=expm[:, lo:lo + 128], in_=sc[:, j * 128:(j + 1) * 128],
                        func=mybir.ActivationFunctionType.Exp)

            for ci, (s, o, w, half) in enumerate(chunks):
                cs = slice(o, o + w)
                # DVE: clip + t2..t5, t7; GpSimd: t6, t8
                nc.vector.tensor_scalar(out=tpow[0][:, cs], in0=kfT[:, cs],
                                        scalar1=CLAMP, scalar2=-CLAMP,
                                        op0=mybir.AluOpType.min,
                                        op1=mybir.AluOpType.max)
                for i in (1, 2, 3, 4):   # t2 = t1*t1 ... t5 = t4*t1
                    nc.vector.tensor_mul(out=tpow[i][:, cs],
                                         in0=tpow[i - 1][:, cs],
                                         in1=tpow[0][:, cs])
                nc.gpsimd.tensor_mul(out=tpow[5][:, cs],     # t6 = t3*t3
                                     in0=tpow[2][:, cs], in1=tpow[2][:, cs])
                nc.vector.tensor_mul(out=tpow[6][:, cs],     # t7 = t4*t3
                                     in0=tpow[3][:, cs], in1=tpow[2][:, cs])
                nc.gpsimd.tensor_mul(out=tpow[7][:, cs],     # t8 = t4*t4
                                     in0=tpow[3][:, cs], in1=tpow[3][:, cs])
                # ACT: shifted tanh columns
                for si in range(len(SHIFTS)):
                    nc.scalar.activation(out=ttanh[si][:, cs],
                                         in_=kfT[:, cs],
                                         func=mybir.ActivationFunctionType.Tanh,
                                         bias=bias_sb[:, si:si + 1])
                # PE: R score matmuls into this half's PSUM rows
                sc = scA if half == 0 else scB
                rows = slice(s * 64, (s + 1) * 64)
                pw = 512 if half == 0 else w
                for r in range(R):
                    nc.tensor.matmul(sc[rows, 0:pw], pslice(s, r),
                                     basis[r][:, cs],
                                     start=False, stop=(r == R - 1))
                if ci == 2:
                    # scA complete (after chunk A1); exp overlaps B chunks.
                    # Emitted after chunk B0's tanh so the ACT queue doesn't
                    # stall B0's tanh-basis matmuls.
                    exp_blocks(scA, 0, range(4))

            exp_blocks(scB, 4, range(4))

            # --- attn transposes + attn @ V --------------------------------
            trps = tc.alloc_tile_pool(name="trps", bufs=2, space="PSUM")
            ops = tc.alloc_tile_pool(name="ops", bufs=1, space="PSUM")
            out_ps = ops.tile([128, VS], F32)
            PT = soft.tile([128, 8, 128], BF16)

            def av_blocks(jrange, base):
                for j in jrange:
                    tr_ps = trps.tile([128, 128], BF16, tag="tr")
                    nc.tensor.transpose(
                        tr_ps, expm[:, base * 128 + j * 128:
                                    base * 128 + (j + 1) * 128], identb_sb)
                    pj = base + j
                    nc.scalar.copy(out=PT[:, pj, :], in_=tr_ps)
                    for s in range(2):
                        if base == 4 and j >= nB[s]:
                            continue
                        nc.tensor.matmul(
                            out_ps[s * 64:(s + 1) * 64, :],
                            PT[:, pj, s * 64:s * 64 + 64],
                            vals_sb[s][:, pj, :],
                            start=(pj == 0),
                            stop=(pj == 4 + nB[s] - 1))

            av_blocks(range(4), 0)
            av_blocks(range(nBmax), 4)

            # row sums off the bf16 exp matrix (fp32 accumulate), off the
            # critical path until the final scale
            stot = soft.tile([128, 1], F32)
            nc.vector.reduce_sum(out=stot, in_=expm,
                                 axis=mybir.AxisListType.X)
            rsum = soft.tile([128, 1], F32)
            nc.vector.reciprocal(out=rsum, in_=stot)

            of = soft.tile([128, VS], F32)
            nc.vector.tensor_scalar_mul(out=of, in0=out_ps, scalar1=rsum)
            nc.sync.dma_start(out=out_d, in_=of)
            ops.release()
            trps.release()
            scps.release()
            warmps.release()

    nc.finalize()
    return nc


def kernel(queries, keys, values, valid_len, Wq, Wk, Ws):
    queries = np.asarray(queries, dtype=np.float32)
    keys = np.asarray(keys, dtype=np.float32)
    values = np.asarray(values, dtype=np.float32)
    Wq = np.asarray(Wq, dtype=np.float32)
    Wk = np.asarray(Wk, dtype=np.float32)
    Ws = np.asarray(Ws, dtype=np.float32)
    vl = np.asarray(valid_len).astype(np.int64)
    assert queries.shape == (B, Q, QS) and keys.shape == (B, K, KS)
    assert values.shape == (B, K, VS) and vl.shape == (B,)

    # Load balance: front-mask => keys < vl masked, so larger vl = less
    # work. slot0 = 8 smallest-vl batches. SPMD => per-slot kept length
    # sized by the slot's min vl (rounded down to 8).
    vlc = np.clip(vl, 0, K - 8)
    order = np.argsort(vlc, kind="stable")
    slots = [order[:NCORES], order[NCORES:]]
    k0 = [int(vlc[s].min()) // 8 * 8 for s in slots]
    L = [K - z for z in k0]
    nblkv = [(Ls + 127) // 128 for Ls in L]

    nc = _build(L, nblkv)

    # host-side projections (exact) + per-q basis weights
    qf = (queries.reshape(B * Q, QS) @ Wq).reshape(B, Q, H)
    kf = (keys.reshape(B * K, KS) @ Wk).reshape(B, K, H).astype(NP_BF16)
    qgrid, Wt = _fit_tables()
    qv = np.clip(qf, qgrid[0], qgrid[-1])
    # P[b, r, h, q] = Ws_h * w_{r+1}(qf[b, q, h])  (col 0 = dropped const)
    wr = np.stack([np.interp(qv, qgrid, Wt[:, r + 1]) for r in range(R)],
                  axis=1)                               # (B, R, Q, H)
    P = (Ws[None, None, None, :] * wr).transpose(0, 1, 3, 2)  # (B,R,H,Q)
    P = np.ascontiguousarray(P).astype(NP_BF16)

    ident = np.eye(128, dtype=NP_BF16)
    onesAB = np.zeros((2, 128), dtype=NP_BF16)
    onesAB[0, 0:64] = 1
    onesAB[1, 64:128] = 1

    chunks = [(0, 0, 512), (1, 0, 512), (0, 512, L[0] - 512),
              (1, 512, L[1] - 512)]

    in_maps = []
    for core in range(NCORES):
        m = {"identb": ident, "onesAB": onesAB}
        kfb = []
        Pmat = np.zeros((128, 2 * R * 64), dtype=NP_BF16)
        maskAB = np.zeros((2, 1024), dtype=NP_BF16)
        for s in range(2):
            b = int(slots[s][core])
            kfb.append(kf[b, k0[s]:, :].T)              # [128, L[s]] bf16
            Pmat[:, s * R * 64:(s + 1) * R * 64] = \
                P[b].transpose(1, 0, 2).reshape(H, R * Q)
            # mask: scA col c = key k0s+c, masked while < vl_b;
            # scB col c = key k0s+512+c, garbage for c >= L_s-512
            nm = int(vl[b]) - k0[s]
            if nm > 0:
                maskAB[s, 0:nm] = NEG
            maskAB[s, 512 + (L[s] - 512):1024] = NEG
            vpad = np.zeros((nblkv[s] * 128, VS), dtype=NP_BF16)
            nreal = K - k0[s]
            vpad[0:nreal] = values[b, k0[s]:, :].astype(NP_BF16)
            m[f"values{s}"] = np.ascontiguousarray(
                vpad.reshape(nblkv[s], 128, VS))
        kfT4 = np.zeros((4, 128, 512), dtype=NP_BF16)
        for ci, (s, o, w) in enumerate(chunks):
            kfT4[ci, :, 0:w] = kfb[s][:, o:o + w]
        m["kfT4"] = kfT4
        m["Pmat"] = Pmat
        m["maskAB"] = maskAB
        in_maps.append(m)

    res = run_bass_kernel_spmd(nc, in_maps, core_ids=list(range(NCORES)),
                               trace=False)
    global LAST_RESULT
    LAST_RESULT = res

    out = np.empty((B, Q, VS), dtype=np.float32)
    for core in range(NCORES):
        o = res.results[core]["out"]  # [128, VS]
        for s in range(2):
            b = int(slots[s][core])
            out[b] = o[s * 64:(s + 1) * 64, :]
    return out


# revision 9
# speedup vs baseline: 1.1985x; 1.1985x over previous
"""Additive attention (nn_AdditiveAttention) Bass kernel for 8 TRN2 NeuronCores.

Reference computation (B=16, Q=64, K=1024, QS=KS=VS=256, H=128):
    q = queries @ Wq                      # (B,Q,H)
    k = keys @ Wk                         # (B,K,H)
    feat = tanh(q[:,:,None,:] + k[:,None,:,:])   # (B,Q,K,H)
    scores = feat @ Ws                    # (B,Q,K)
    scores = where(arange(K) >= valid_len[b], scores, -1e6)
    out = softmax(scores) @ values        # (B,Q,VS)

Strategy: replace the elementwise tanh over (B,Q,K,H) with a rank-R
separable approximation

    tanh(q + k) ~= sum_r w_r(q) * g_r(k)

where the k-side basis g_r is device-computable (clipped powers k^1..k^8
on DVE, shifted tanh(k+s) on ACT) and the q-side weights w_r are
evaluated EXACTLY on the host (per-q L2 fit from a lookup table). Then

    scores[q,k] = sum_h Ws_h tanh(qf+kf) ~= sum_r <P_r[:,q], g_r(kf)[:,k]>_h

with P_r[h,q] = Ws_h * w_r(qf[h,q]) shipped bf16 from host: R matmuls
contracting over H=128 replace the 134M-element tanh (ACT-bound in the
old kernel). A constant basis column is included in the fit but dropped
on device: it shifts each row's scores by a per-q constant, which
softmax cancels.

Per core (data-parallel over batch, 2 batches/core as slot0 rows 0-63
and slot1 rows 64-127, valid_len-aware skipping of masked leading keys):
  - host: qf, kf projections (exact fp32 GEMM), P_r tables, masks,
    values shuffling to partition-major, all bf16 casts.
  - device DVE: clip kf + 7 chained multiplies for k^2..k^8.
  - device ACT: 2 shifted tanh basis columns, blockwise exp with fused
    row-sum accumulation, attn-transpose PSUM->SBUF copies.
  - device PE (kept dense: idle gaps reset the 2.4 GHz p-state ramp):
    warmup, mask-seed matmuls, R score matmuls per (slot, 512-col half)
    into fp32 PSUM, attn transposes, attn @ V.
  - DMA queues: scalar=kfT slot0, tensor=kfT slot1, sync=Pmat+values0,
    gpsimd=small consts + values1, vector=output.
"""

import sys

if "/opt/trn_rl_repo" not in sys.path:
    sys.path.insert(0, "/opt/trn_rl_repo")

import ml_dtypes
import numpy as np

import concourse.bass as bass  # noqa: F401
import concourse.mybir as mybir
import concourse.tile as tile
from concourse import bacc
from concourse.bass_utils import run_bass_kernel_spmd

LAST_RESULT = None  # BassKernelResults of the most recent kernel() call

B, Q, K = 16, 64, 1024
QS = KS = VS = 256
H = 128
NCORES = 8
NEG = -1.0e6
F32 = mybir.dt.float32
BF16 = mybir.dt.bfloat16
NP_BF16 = ml_dtypes.bfloat16

NPOW = 8                      # clipped powers k^1..k^NPOW (DVE)
SHIFTS = (-2.5, 2.5)          # tanh(k + s) basis columns (ACT)
R = NPOW + len(SHIFTS)        # device basis size (const col dropped)
CLAMP = 4.5
NWARM = 7                     # PE clock-ramp warmup matmuls

_FIT_CACHE = None


def _bf(x):
    return np.asarray(x, np.float32).astype(NP_BF16).astype(np.float32)


def _basis_cols(kv):
    """[len(kv), R+1] host model of the device basis (col 0 = const),
    including the bf16 rounding of the device compute chain."""
    kv = np.asarray(kv, np.float32)
    t1 = _bf(np.clip(kv, -CLAMP, CLAMP))
    cols = [np.ones_like(kv), t1]
    cur = t1
    for _ in range(2, NPOW + 1):
        cur = _bf(cur * t1)
        cols.append(cur)
    for s in SHIFTS:
        cols.append(_bf(np.tanh(kv + s)))
    return np.stack(cols, -1).astype(np.float32)


def _fit_tables():
    """Per-q weight lookup table (qgrid, Wt[nq, R+1]) for the L2 fit of
    tanh(q+k) onto the device basis, under a Gaussian+floor k-weight."""
    global _FIT_CACHE
    if _FIT_CACHE is not None:
        return _FIT_CACHE
    kgrid = np.linspace(-5.6, 5.6, 2241)
    wg = np.exp(-kgrid ** 2 / 2)
    wg /= wg.sum()
    wg += 0.01 / len(kgrid)
    qgrid = np.linspace(-5.2, 5.2, 2081)
    Gk = _basis_cols(kgrid)
    sw = np.sqrt(wg)[:, None]
    gram = (Gk * sw).T @ (Gk * sw) + 1e-6 * np.eye(R + 1)
    T = np.tanh(qgrid[:, None] + kgrid[None, :])
    bm = (T * wg[None, :]) @ Gk
    Wt = np.linalg.solve(gram, bm.T).T
    _FIT_CACHE = (qgrid, Wt)
    return _FIT_CACHE


def _build(L, nblkv):
    """Per-core Bass graph. L/nblkv: 2-element lists of per-slot kept key
    length (multiple of 8, > 512) and value block count (ceil(L/128))."""
    nc = bacc.Bacc("TRN2", target_bir_lowering=False, debug=False,
                   num_devices=NCORES)
    L0, L1 = L
    LT = L0 + L1
    nB = [n - 4 for n in nblkv]
    nBmax = max(nB)

    # chunk meta: (slot, kfT col offset, width, psum half)
    chunks = [
        (0, 0, 512, 0),
        (1, L0, 512, 0),
        (0, 512, L0 - 512, 1),
        (1, L0 + 512, L1 - 512, 1),
    ]

    inp = {
        "kfT0": nc.dram_tensor("kfT0", [128, L0], BF16,
                               kind="ExternalInput").ap(),
        "kfT1": nc.dram_tensor("kfT1", [128, L1], BF16,
                               kind="ExternalInput").ap(),
        "Pmat": nc.dram_tensor("Pmat", [128, 2 * R * 64], BF16,
                               kind="ExternalInput").ap(),
        "maskAB": nc.dram_tensor("maskAB", [2, 1024], BF16,
                                 kind="ExternalInput").ap(),
        "onesAB": nc.dram_tensor("onesAB", [2, 128], BF16,
                                 kind="ExternalInput").ap(),
        "identb": nc.dram_tensor("identb", [128, 128], BF16,
                                 kind="ExternalInput").ap(),
        "values0": nc.dram_tensor("values0", [128, nblkv[0], VS], BF16,
                                  kind="ExternalInput").ap(),
        "values1": nc.dram_tensor("values1", [nblkv[1], 128, VS], BF16,
                                  kind="ExternalInput").ap(),
    }
    out_d = nc.dram_tensor("out", [128, VS], F32, kind="ExternalOutput").ap()

    with tile.TileContext(nc) as tc:
        with (
            tc.tile_pool(name="consts", bufs=1) as consts,
            tc.tile_pool(name="feat", bufs=1) as feat,
            tc.tile_pool(name="vals", bufs=1) as vals,
            tc.tile_pool(name="soft", bufs=1) as soft,
        ):
            # --- early const setup on GpSimd (earliest-starting engine) ----
            warm_sb = consts.tile([128, 512], BF16)
            nc.gpsimd.memset(warm_sb, 0.5)
            bias_sb = consts.tile([128, max(2, len(SHIFTS))], F32)
            for si, sh in enumerate(SHIFTS):
                nc.gpsimd.memset(bias_sb[:, si:si + 1], float(sh))
            dum_sb = consts.tile([128, 1], BF16)
            ones_sb = consts.tile([2, 128], BF16)
            nc.gpsimd.dma_start(out=ones_sb, in_=inp["onesAB"])
            mask_sb = consts.tile([2, 1024], BF16)
            nc.gpsimd.dma_start(out=mask_sb, in_=inp["maskAB"])
            identb_sb = consts.tile([128, 128], BF16)
            nc.gpsimd.dma_start(out=identb_sb, in_=inp["identb"])

            # ACT activation-table preload while DMAs land
            nc.scalar.activation(out=dum_sb, in_=bias_sb[:, 0:1],
                                 func=mybir.ActivationFunctionType.Tanh,
                                 bias=bias_sb[:, 0:1])
            nc.scalar.activation(out=dum_sb, in_=bias_sb[:, 0:1],
                                 func=mybir.ActivationFunctionType.Exp)

            # --- bulk DMAs, one queue each ---------------------------------
            kfT = feat.tile([128, LT], BF16)
            nc.scalar.dma_start(out=kfT[:, 0:L0], in_=inp["kfT0"])
            nc.sync.dma_start(out=kfT[:, L0:LT], in_=inp["kfT1"])
            p_sb = consts.tile([128, 2 * R * 64], BF16)
            nc.sync.dma_start(out=p_sb, in_=inp["Pmat"])
            v0_sb = vals.tile([128, nblkv[0], VS], BF16)
            nc.sync.dma_start(out=v0_sb, in_=inp["values0"])
            v1_sb = vals.tile([128, nblkv[1], VS], BF16)
            for j in range(nblkv[1]):
                nc.gpsimd.dma_start(out=v1_sb[:, j, :], in_=inp["values1"][j])
            vals_sb = [v0_sb, v1_sb]

            # PE warmup (p-state ramp) as soon as warm_sb is set; must flow
            # gaplessly into the real matmul stream
            warmps = tc.alloc_tile_pool(name="warmps", bufs=1, space="PSUM")
            warm_ps = warmps.tile([128, 512], F32)
            for _ in range(NWARM):
                nc.tensor.matmul(warm_ps, warm_sb[:, 0:128], warm_sb,
                                 start=True, stop=True)

            def pslice(s, r):
                o = (s * R + r) * 64
                return p_sb[:, o:o + 64]

            # --- basis + scores --------------------------------------------
            tpow = [feat.tile([128, LT], BF16, name=f"t{i + 1}")
                    for i in range(NPOW)]
            ttanh = [feat.tile([128, LT], BF16, name=f"tanh{si}")
                     for si in range(len(SHIFTS))]
            basis = tpow + ttanh

            scps = tc.alloc_tile_pool(name="scps", bufs=1, space="PSUM")
            scA = scps.tile([128, 512], F32, tag="scA")
            scB = scps.tile([128, 512], F32, tag="scB")
            nc.tensor.matmul(scA, ones_sb, mask_sb[:, 0:512], start=True,
                             stop=False)
            nc.tensor.matmul(scB, ones_sb, mask_sb[:, 512:1024], start=True,
                             stop=False)

            expm = soft.tile([128, 1024], BF16)
            sums = soft.tile([128, 8], F32)

            def exp_blocks(sc, base, jrange):
                for j in jrange:
                    lo = base * 128 + j * 128
                    si = (0 if base == 0 else 4) + j
                    nc.scalar.activation(
                        out=expm[:, lo:lo + 128],
                        in_=sc[:, j * 128:(j + 1) * 128],
                        func=mybir.ActivationFunctionType.Exp,
                        accum_out=sums[:, si:si + 1])

            for ci, (s, o, w, half) in enumerate(chunks):
                cs = slice(o, o + w)
                nc.vector.tensor_scalar(out=tpow[0][:, cs], in0=kfT[:, cs],
                                        scalar1=CLAMP, scalar2=-CLAMP,
                                        op0=mybir.AluOpType.min,
                                        op1=mybir.AluOpType.max)
                for i in range(1, NPOW):
                    nc.vector.tensor_mul(out=tpow[i][:, cs],
                                         in0=tpow[i - 1][:, cs],
                                         in1=tpow[0][:, cs])
                for si in range(len(SHIFTS)):
                    nc.scalar.activation(out=ttanh[si][:, cs],
                                         in_=kfT[:, cs],
                                         func=mybir.ActivationFunctionType.Tanh,
                                         bias=bias_sb[:, si:si + 1])
                sc = scA if half == 0 else scB
                rows = slice(s * 64, (s + 1) * 64)
                pw = 512 if half == 0 else w
                for r in range(R):
                    nc.tensor.matmul(sc[rows, 0:pw], pslice(s, r),
                                     basis[r][:, cs],
                                     start=False, stop=(r == R - 1))
                if ci == 2:
                    # scA complete; exp overlaps the B chunks. Emitted after
                    # chunk B0's tanh so ACT doesn't stall B0's matmuls.
                    exp_blocks(scA, 0, range(4))

            # --- attn transposes + attn @ V --------------------------------
            trps = tc.alloc_tile_pool(name="trps", bufs=2, space="PSUM")
            ops = tc.alloc_tile_pool(name="ops", bufs=1, space="PSUM")
            out_ps = ops.tile([128, VS], F32)
            PT = soft.tile([128, 8, 128], BF16)

            def av_blocks(jrange, base):
                for j in jrange:
                    tr_ps = trps.tile([128, 128], BF16, tag="tr")
                    nc.tensor.transpose(
                        tr_ps, expm[:, base * 128 + j * 128:
                                    base * 128 + (j + 1) * 128], identb_sb)
                    pj = base + j
                    nc.scalar.copy(out=PT[:, pj, :], in_=tr_ps)
                    for s in range(2):
                        if base == 4 and j >= nB[s]:
                            continue
                        nc.tensor.matmul(
                            out_ps[s * 64:(s + 1) * 64, :],
                            PT[:, pj, s * 64:s * 64 + 64],
                            vals_sb[s][:, pj, :],
                            start=(pj == 0),
                            stop=(pj == 4 + nB[s] - 1))

            av_blocks(range(4), 0)
            exp_blocks(scB, 4, range(4))
            av_blocks(range(nBmax), 4)

            stot = soft.tile([128, 1], F32)
            nc.vector.reduce_sum(out=stot, in_=sums,
                                 axis=mybir.AxisListType.X)
            rsum = soft.tile([128, 1], F32)
            nc.vector.reciprocal(out=rsum, in_=stot)

            of = soft.tile([128, VS], F32)
            nc.vector.tensor_scalar_mul(out=of, in0=out_ps, scalar1=rsum)
            nc.scalar.dma_start(out=out_d, in_=of)
            ops.release()
            trps.release()
            scps.release()
            warmps.release()

    nc.finalize()
    return nc


def kernel(queries, keys, values, valid_len, Wq, Wk, Ws):
    queries = np.asarray(queries, dtype=np.float32)
    keys = np.asarray(keys, dtype=np.float32)
    values = np.asarray(values, dtype=np.float32)
    Wq = np.asarray(Wq, dtype=np.float32)
    Wk = np.asarray(Wk, dtype=np.float32)
    Ws = np.asarray(Ws, dtype=np.float32)
    vl = np.asarray(valid_len).astype(np.int64)
    assert queries.shape == (B, Q, QS) and keys.shape == (B, K, KS)
    assert values.shape == (B, K, VS) and vl.shape == (B,)

    # Load balance: front-mask => keys < vl masked, so larger vl = less
    # work. slot0 = 8 smallest-vl batches. SPMD => per-slot kept length
    # sized by the slot's min vl (rounded down to 8).
    vlc = np.clip(vl, 0, K - 8)
    order = np.argsort(vlc, kind="stable")
    slots = [order[:NCORES], order[NCORES:]]
    k0 = [int(vlc[s].min()) // 8 * 8 for s in slots]
    L = [K - z for z in k0]
    nblkv = [(Ls + 127) // 128 for Ls in L]

    nc = _build(L, nblkv)

    # host-side projections (exact) + per-q basis weights
    qf = (queries.reshape(B * Q, QS) @ Wq).reshape(B, Q, H)
    kf = (keys.reshape(B * K, KS) @ Wk).reshape(B, K, H).astype(NP_BF16)
    qgrid, Wt = _fit_tables()
    qv = np.clip(qf, qgrid[0], qgrid[-1])
    # P[b, r, h, q] = Ws_h * w_{r+1}(qf[b, q, h])  (col 0 = dropped const)
    wr = np.stack([np.interp(qv, qgrid, Wt[:, r + 1]) for r in range(R)],
                  axis=1)                               # (B, R, Q, H)
    P = (Ws[None, None, None, :] * wr).transpose(0, 1, 3, 2)  # (B,R,H,Q)
    P = np.ascontiguousarray(P).astype(NP_BF16)

    ident = np.eye(128, dtype=NP_BF16)
    onesAB = np.zeros((2, 128), dtype=NP_BF16)
    onesAB[0, 0:64] = 1
    onesAB[1, 64:128] = 1

    in_maps = []
    for core in range(NCORES):
        m = {"identb": ident, "onesAB": onesAB}
        Pmat = np.zeros((128, 2 * R * 64), dtype=NP_BF16)
        maskAB = np.zeros((2, 1024), dtype=NP_BF16)
        for s in range(2):
            b = int(slots[s][core])
            m[f"kfT{s}"] = np.ascontiguousarray(kf[b, k0[s]:, :].T)
            Pmat[:, s * R * 64:(s + 1) * R * 64] = \
                P[b].transpose(1, 0, 2).reshape(H, R * Q)
            # mask: scA col c = key k0s+c, masked while < vl_b;
            # scB col c = key k0s+512+c, garbage for c >= L_s-512
            nm = int(vl[b]) - k0[s]
            if nm > 0:
                maskAB[s, 0:nm] = NEG
            maskAB[s, 512 + (L[s] - 512):1024] = NEG
            vpad = np.zeros((nblkv[s] * 128, VS), dtype=NP_BF16)
            nreal = K - k0[s]
            vpad[0:nreal] = values[b, k0[s]:, :].astype(NP_BF16)
            vb = vpad.reshape(nblkv[s], 128, VS)
            if s == 0:
                # partition-major for a single contiguous-row DMA
                m["values0"] = np.ascontiguousarray(vb.transpose(1, 0, 2))
            else:
                m["values1"] = np.ascontiguousarray(vb)
        m["Pmat"] = Pmat
        m["maskAB"] = maskAB
        in_maps.append(m)

    res = run_bass_kernel_spmd(nc, in_maps, core_ids=list(range(NCORES)),
                               trace=False)
    global LAST_RESULT
    LAST_RESULT = res

    out = np.empty((B, Q, VS), dtype=np.float32)
    for core in range(NCORES):
        o = res.results[core]["out"]  # [128, VS]
        for s in range(2):
            b = int(slots[s][core])
            out[b] = o[s * 64:(s + 1) * 64, :]
    return out
